# revision 15
# baseline (speedup 1.0000x reference)
"""Trainium2 Bass kernel for nn_CSSMBlock: conv residual block + LayerNorm + Mamba
selective scan on (2, 64, 128, 128), distributed over 8 NeuronCores.

Sharding: sequence-parallel. Core k handles sample b = k//4, image rows
[seg*32, seg*32+32) where seg = k%4 (4096 sequence positions each). The Mamba
scan runs in two phases around an AllGather of per-segment boundary states:
  phase 1: local scan from h=0 (tensor_tensor_scan per state index s)
           -> segment end-state G[d,s] and decay-sum dS[d]
  AllGather(G|dS) -> each core combines its predecessors' summaries into its
           true incoming state H_in
  phase 2: re-scan with initial=H_in, project with C (PE-accumulated y),
           gate with silu(z), out-project, final residual combine.
dB = (dt*u) outer B is spilled to DRAM in phase 1 and reloaded in phase 2.
LayerNorm is folded into in_proj: conv_out is normalized in place (64
partitions) before the in_proj matmul; gain/bias fold into the weights.
"""

import numpy as np

import concourse.bass as bass
import concourse.bacc as bacc
import concourse.mybir as mybir
import concourse.tile as tile
from concourse.bass_utils import run_bass_kernel_spmd

F32 = mybir.dt.float32
F32R = mybir.dt.float32r
AF = mybir.ActivationFunctionType
OP = mybir.AluOpType

B, C, H, W = 2, 64, 128, 128
DIN, DST, DTR, DCONV = 128, 16, 4, 4
LN_EPS = 1e-5
N_CORES = 8
SEGS = 4
ROWS = H // SEGS          # 32
LC = ROWS * W             # 4096
XROWS = ROWS + 5          # 37
C1ROWS = ROWS + 3         # 35
COROWS = ROWS + 1         # 33
WP = W + 2                # 130
TSC = 2048
NSL = 512

NIMG = XROWS * WP         # 4810
NC1 = C1ROWS * WP         # 4550
LT = COROWS * W           # 4224

_cached = {}


def _r(ap):
    if ap.dtype == F32R:
        return ap
    return ap.bitcast(F32R)


def _build():
    nc = bacc.Bacc("TRN2", target_bir_lowering=False, debug=False,
                   num_devices=N_CORES)

    def din(name, shape, dt=F32):
        return nc.dram_tensor(name, list(shape), dt, kind="ExternalInput").ap()

    xs = din("xs", (C, XROWS, W), F32R)
    w1t = din("w1t", (C, 9 * C), F32R)
    w2t = din("w2t", (C, 9 * C), F32R)
    cb1 = din("cb1", (C, 1))
    cb2 = din("cb2", (C, 1))
    ident64 = din("ident64", (C, C), F32R)
    ident128 = din("ident128", (DIN, DIN), F32R)
    onesab = din("onesab", (2 * C, 2), F32R)
    wgt = din("wgt", (2 * C, 2 * DIN), F32R)
    xpwt = din("xpwt", (DIN, DTR + 2 * DST), F32R)
    dtwt = din("dtwt", (32 + DTR, DIN), F32R)
    dtb = din("dtb", (DIN, 1))
    cw = din("cw", (DIN, DCONV))
    bprime = din("bprime", (DIN, 1))
    c2z = din("c2z", (DIN, 1))
    opt_w = din("opt_w", (DIN, C), F32R)
    a_mat = din("a_mat", (DIN, DST))
    dvec = din("dvec", (DIN, 1))
    alpha = din("alpha", (DIN, N_CORES))
    maskc = din("maskc", (DIN, 1))
    halo_fill = din("halo_fill", (DIN, 3))
    c1m = din("c1m", (C, C1ROWS))

    y_out = nc.dram_tensor("y_out", [C, LC], F32, kind="ExternalOutput").ap()

    cc_in = nc.dram_tensor("cc_in", [DIN, DST + 1], F32).ap()
    cc_out = nc.dram_tensor("cc_out", [N_CORES * DIN, DST + 1], F32,
                            addr_space="Shared").ap()
    db_spill = nc.dram_tensor("db_spill", [DST, DIN, LC], F32).ap()

    with tile.TileContext(nc, trace_sim=False) as tc:
        cst = tc.alloc_tile_pool(name="cst", bufs=1)
        seq = tc.alloc_tile_pool(name="seq", bufs=1)

        def load(ap_in, p, f, nm, dt=F32):
            t = cst.tile([p, f], dt, name=nm)
            nc.sync.dma_start(t[:], ap_in[:])
            return t

        w1t_s = load(w1t, C, 9 * C, "w1t_s", F32R)
        w2t_s = load(w2t, C, 9 * C, "w2t_s", F32R)
        cb1_s = load(cb1, C, 1, "cb1_s")
        cb2_s = load(cb2, C, 1, "cb2_s")
        id64_s = load(ident64, C, C, "id64_s", F32R)
        id128_s = load(ident128, DIN, DIN, "id128_s", F32R)
        onesab_s = load(onesab, 2 * C, 2, "onesab_s", F32R)
        wgt_s = load(wgt, 2 * C, 2 * DIN, "wgt_s", F32R)
        xpwt_s = load(xpwt, DIN, DTR + 2 * DST, "xpwt_s", F32R)
        dtwt_s = load(dtwt, 32 + DTR, DIN, "dtwt_s", F32R)
        dtb_s = load(dtb, DIN, 1, "dtb_s")
        cw_s = load(cw, DIN, DCONV, "cw_s")
        bprime_s = load(bprime, DIN, 1, "bprime_s")
        c2z_s = load(c2z, DIN, 1, "c2z_s")
        opt_s = load(opt_w, DIN, C, "opt_s", F32R)
        a_s = load(a_mat, DIN, DST, "a_s")
        dvec_s = load(dvec, DIN, 1, "dvec_s")
        alpha_s = load(alpha, DIN, N_CORES, "alpha_s")
        maskc_s = load(maskc, DIN, 1, "maskc_s")
        halo_s = load(halo_fill, DIN, 3, "halo_s")
        c1m_s = load(c1m, C, C1ROWS, "c1m_s")

        u_t = seq.tile([DIN, LC], F32R, name="u_t")        # u, later y
        dt_t = seq.tile([DIN, LC], F32, name="dt_t")
        zs_t = seq.tile([DIN, LC], F32, name="zs_t")
        co_t = seq.tile([C, LC], F32, name="co_t")
        # pack: row 0 = broadcast staging; rows 32:36 dt_r; 36:52 B; 52:68 C
        pack = seq.tile([68, LC], F32R, name="pack")
        gcat = seq.tile([DIN, DST + 1], F32, name="gcat")
        gat = seq.tile([DIN, N_CORES * (DST + 1)], F32, name="gat")
        hin = seq.tile([DIN, DST], F32, name="hin")
        ones128 = seq.tile([DIN, 1], F32, name="ones128")
        eps1 = seq.tile([1, 1], F32, name="eps1")
        nc.vector.memset(ones128[:], 1.0)
        nc.vector.memset(eps1[:], LN_EPS)

        # ---------------- front ----------------
        with tc.tile_pool(name="img", bufs=1) as img, \
             tc.tile_pool(name="fpsum", bufs=1, space="PSUM") as fpsum:
            xpg = img.tile([C, NIMG + 2], F32R, name="xpg")
            c1g = img.tile([C, NC1 + 2], F32R, name="c1g")
            stk = img.tile([2 * C, LT], F32R, name="stk")
            xpart = img.tile([DIN, LT], F32, name="xpart")

            xg = xpg[:, 1:NIMG + 1].rearrange("p (r c) -> p r c", r=XROWS, c=WP)
            nc.vector.memset(xpg[:, 0:1].bitcast(F32), 0.0)
            nc.vector.memset(xpg[:, NIMG + 1:NIMG + 2].bitcast(F32), 0.0)
            nc.vector.memset(xg[:, :, 0:1].bitcast(F32), 0.0)
            nc.vector.memset(xg[:, :, WP - 1:WP].bitcast(F32), 0.0)
            nc.sync.dma_start(xg[:, :, 1:W + 1], xs[:])

            # conv1 + relu (c1 grid rows 0..34; c1 row i <-> x grid row i+1)
            for sl0 in range(0, NC1, NSL):
                n = min(NSL, NC1 - sl0)
                ps = fpsum.tile([C, NSL], F32, name="cps1", tag="cps", bufs=2)
                for tap in range(9):
                    dy, dx = tap // 3 - 1, tap % 3 - 1
                    off = sl0 + (dy + 1) * WP + dx + 1
                    nc.tensor.matmul(
                        ps[:, :n], _r(w1t_s[:, tap * C:(tap + 1) * C]),
                        _r(xpg[:, off:off + n]), start=(tap == 0), stop=(tap == 8))
                nc.scalar.activation(c1g[:, 1 + sl0:1 + sl0 + n], ps[:, :n],
                                     AF.Relu, bias=cb1_s[:])
            nc.vector.memset(c1g[:, 0:1].bitcast(F32), 0.0)
            nc.vector.memset(c1g[:, NC1 + 1:NC1 + 2].bitcast(F32), 0.0)
            c1v = c1g[:, 1:NC1 + 1].rearrange("p (r c) -> p r c", r=C1ROWS, c=WP)
            nc.vector.memset(c1v[:, :, 0:1].bitcast(F32), 0.0)
            nc.vector.memset(c1v[:, :, WP - 1:WP].bitcast(F32), 0.0)
            # zero conv1 rows outside the image (conv2 SAME padding)
            mbc = c1m_s[:].rearrange("p (r o) -> p r o", o=1)
            nc.vector.tensor_tensor(c1v[:], c1v[:],
                                    mbc.broadcast_to((C, C1ROWS, WP)), OP.mult)

            # conv2 + residual, 3 rows per psum tile, strided ACT drops pads
            skv = stk[0:C, :].rearrange("p (r c) -> p r c", r=COROWS, c=W)
            for j in range(0, COROWS, 3):
                p0 = j * WP
                n = 3 * WP
                ps = fpsum.tile([C, 3 * WP], F32, name="cps2", tag="cps", bufs=2)
                for tap in range(9):
                    dy, dx = tap // 3, tap % 3 - 1
                    off = p0 + dy * WP + dx + 1
                    nc.tensor.matmul(
                        ps[:], _r(w2t_s[:, tap * C:(tap + 1) * C]),
                        _r(c1g[:, off:off + n]), start=(tap == 0), stop=False)
                nc.tensor.matmul(
                    ps[:], _r(id64_s[:]),
                    _r(xpg[:, p0 + 2 * WP + 1:p0 + 2 * WP + 1 + n]),
                    start=False, stop=True)
                psv = ps[:].rearrange("p (r c) -> p r c", r=3, c=WP)
                nc.scalar.activation(skv[:, j:j + 3, :], psv[:, :, 1:W + 1],
                                     AF.Identity, bias=cb2_s[:])

            # keep raw conv_out (real cols) for the tail
            nc.vector.tensor_copy(co_t[:], stk[0:C, W:W + LC])
            # copy co to partitions 64..127, square in place at 0..63
            nc.sync.dma_start(stk[C:2 * C, :], stk[0:C, :])
            nc.scalar.activation(stk[0:C, :], stk[0:C, :], AF.Square)

            # stats: sums (of co, rows 64:128) and sqsums (rows 0:64), both on
            # partition 0 via two 1-column matmuls; lane-0 scalar chain
            rm_t = img.tile([1, LT], F32, name="rm_t", tag="c1g")
            rs_t = img.tile([1, LT], F32, name="rs_t", tag="xpart")
            for sl0 in range(0, LT, NSL):
                n = min(NSL, LT - sl0)
                psa = fpsum.tile([1, NSL], F32, name="psa", tag="sps", bufs=1)
                psb = fpsum.tile([1, NSL], F32, name="psb", tag="spsb", bufs=1)
                nc.tensor.matmul(psa[:, :n], _r(onesab_s[:, 0:1]),
                                 _r(stk[:, sl0:sl0 + n]), start=True, stop=True)
                nc.tensor.matmul(psb[:, :n], _r(onesab_s[:, 1:2]),
                                 _r(stk[:, sl0:sl0 + n]), start=True, stop=True)
                sm = rm_t[:, sl0:sl0 + n]
                rv = rs_t[:, sl0:sl0 + n]
                nc.scalar.activation(sm, psa[:, :n], AF.Copy)     # sums
                nc.vector.scalar_tensor_tensor(rv, sm, -1.0 / C, sm,
                                               OP.mult, OP.mult)  # -sums^2/64
                nc.vector.tensor_tensor(rv, rv, psb[:, :n], OP.add)  # 64*var
                nc.scalar.activation(rv, rv, AF.Sqrt, bias=eps1[:],
                                     scale=1.0 / C)
                nc.vector.reciprocal(rv, rv)                      # rs
                nc.vector.tensor_tensor(sm, rv, sm, OP.mult)      # rm = rs*sums

            # normalize co in place at partitions 64..127: co*rs - rm/64
            bct = img.tile([2 * C, LT], F32, name="bct", tag="xpg")
            nc.gpsimd.partition_broadcast(bct[:], rs_t[:])
            nc.vector.tensor_tensor(stk[C:2 * C, :], stk[C:2 * C, :],
                                    bct[C:2 * C, :], OP.mult)
            nc.gpsimd.partition_broadcast(bct[:], rm_t[:])
            nc.vector.scalar_tensor_tensor(stk[C:2 * C, :], bct[C:2 * C, :],
                                           -1.0 / C, stk[C:2 * C, :],
                                           OP.mult, OP.add)

            # in_proj on normalized conv_out (gain/bias folded into wgt/biases)
            for half in range(2):
                for sl0 in range(0, LT, NSL):
                    n = min(NSL, LT - sl0)
                    ps = fpsum.tile([DIN, NSL], F32, name="pps", tag="pps",
                                    bufs=2)
                    nc.tensor.matmul(
                        ps[:, :n],
                        _r(wgt_s[C:2 * C, half * DIN:(half + 1) * DIN]),
                        _r(stk[C:2 * C, sl0:sl0 + n]), start=True, stop=True)
                    if half == 0:
                        nc.scalar.activation(xpart[:, sl0:sl0 + n], ps[:, :n],
                                             AF.Identity, bias=0.0)
                    else:
                        if sl0 + n <= W:
                            continue
                        lo = max(sl0, W)
                        nc.scalar.activation(zs_t[:, lo - W:sl0 + n - W],
                                             ps[:, lo - sl0:n], AF.Silu,
                                             bias=c2z_s[:])

            # seg-0 halo handling: xpart[:, W-3:W] = xpart*mask + halo_fill
            nc.vector.scalar_tensor_tensor(
                xpart[:, W - 3:W], xpart[:, W - 3:W], maskc_s[:], halo_s[:],
                OP.mult, OP.add)

            # depthwise causal conv1d into u_t, then silu in place
            nc.vector.tensor_scalar(u_t[:], xpart[:, W - 3:W - 3 + LC],
                                    cw_s[:, 0:1], None, OP.mult)
            for k in range(1, DCONV):
                nc.vector.scalar_tensor_tensor(
                    u_t[:], xpart[:, W - 3 + k:W - 3 + k + LC], cw_s[:, k:k + 1],
                    u_t[:], OP.mult, OP.add)
            nc.scalar.activation(u_t[:], u_t[:], AF.Silu, bias=bprime_s[:])

            # x_proj -> pack rows 4:40
            for sl0 in range(0, LC, NSL):
                ps = fpsum.tile([DTR + 2 * DST, NSL], F32, name="xps",
                                tag="xps", bufs=2)
                nc.tensor.matmul(ps[:], _r(xpwt_s[:]), _r(u_t[:, sl0:sl0 + NSL]),
                                 start=True, stop=True)
                xst = img.tile([DTR + 2 * DST, NSL], F32R, name="xst", tag="xst",
                               bufs=2)
                nc.scalar.activation(xst[:], ps[:], AF.Copy)
                nc.sync.dma_start(pack[32:68, sl0:sl0 + NSL], xst[:])

            # dt = softplus = ln(1 + exp(dt_proj + b))
            for sl0 in range(0, LC, NSL):
                ps = fpsum.tile([DIN, NSL], F32, name="dps", tag="pps", bufs=2)
                nc.tensor.matmul(ps[:], _r(dtwt_s[32:32 + DTR, :]),
                                 _r(pack[32:32 + DTR, sl0:sl0 + NSL]),
                                 start=True, stop=True)
                nc.scalar.activation(ps[:], ps[:], AF.Exp, bias=dtb_s[:])
                nc.scalar.activation(dt_t[:, sl0:sl0 + NSL], ps[:], AF.Ln,
                                     bias=ones128[:])

            nc.vector.tensor_reduce(gcat[:, DST:DST + 1], dt_t[:],
                                    mybir.AxisListType.X, OP.add)

        # ---------------- scan ----------------
        with tc.tile_pool(name="scan", bufs=1) as scn:
            w_t = scn.tile([DIN, LC], F32, name="w_t")
            nc.vector.tensor_tensor(w_t[:], dt_t[:], u_t[:], OP.mult)

            # phase 1: local scan, G_seg extraction, dB spill
            for s in range(DST):
                nc.sync.dma_start(pack[0:1, :], pack[36 + s:37 + s, :])
                bcb = scn.tile([DIN, LC], F32R, name="bcb", tag="bcb")
                nc.gpsimd.partition_broadcast(bcb[:], pack[0:1, :])
                for ht in range(LC // TSC):
                    t0 = ht * TSC
                    da = scn.tile([DIN, TSC], F32, name="da", tag="da", bufs=2)
                    nc.scalar.activation(da[:], dt_t[:, t0:t0 + TSC], AF.Exp,
                                         scale=a_s[:, s:s + 1])
                    db = scn.tile([DIN, TSC], F32, name="db", tag="db", bufs=2)
                    nc.vector.tensor_tensor(db[:], w_t[:, t0:t0 + TSC],
                                            bcb[:, t0:t0 + TSC], OP.mult)
                    nc.sync.dma_start(db_spill[s, :, t0:t0 + TSC], db[:])
                    h1 = scn.tile([DIN, TSC], F32, name="h1", tag="h1", bufs=2)
                    init = 0.0 if ht == 0 else prev[:, TSC - 1:TSC]
                    nc.vector.tensor_tensor_scan(h1[:], da[:], db[:], init,
                                                 OP.mult, OP.add)
                    prev = h1
                nc.vector.tensor_copy(gcat[:, s:s + 1], prev[:, TSC - 1:TSC])

            # AllGather boundary summaries
            nc.sync.dma_start(cc_in[:], gcat[:])
            nc.gpsimd.collective_compute(
                "AllGather", OP.bypass,
                replica_groups=[list(range(N_CORES))],
                ins=[cc_in[:]], outs=[cc_out[:]])
            gatv = gat[:].rearrange("p (g f) -> p g f", g=N_CORES)
            nc.sync.dma_start(
                gatv[:], cc_out[:].rearrange("(g p) f -> p g f", p=DIN))

            # combine: hin = sum_i alpha_i G_i prod_{k>i} E_k~
            nc.vector.memset(hin[:], 0.0)
            for i in range(N_CORES):
                epre = scn.tile([DIN, DST], F32, name="epre", tag="epre", bufs=2)
                nc.vector.tensor_scalar(epre[:], a_s[:],
                                        gatv[:, i, DST:DST + 1], None, OP.mult)
                nc.scalar.activation(epre[:], epre[:], AF.Exp)
                nc.vector.tensor_scalar(epre[:], epre[:], -1.0, None, OP.add)
                nc.scalar.activation(epre[:], epre[:], AF.Identity,
                                     bias=ones128[:], scale=alpha_s[:, i:i + 1])
                nc.vector.tensor_tensor(hin[:], hin[:], epre[:], OP.mult)
                nc.vector.scalar_tensor_tensor(
                    hin[:], gatv[:, i, 0:DST], alpha_s[:, i:i + 1], hin[:],
                    OP.mult, OP.add)

            # phase 2: true scan + y accumulation in PSUM
            with tc.tile_pool(name="ypp", bufs=1, space="PSUM") as ypp:
                ypsum = ypp.tile([DIN, LC], F32, name="ypsum")
                for s in range(DST):
                    nc.sync.dma_start(pack[0:1, :], pack[52 + s:53 + s, :])
                    bcc = scn.tile([DIN, LC], F32R, name="bcc", tag="bcb")
                    nc.gpsimd.partition_broadcast(bcc[:], pack[0:1, :])
                    for ht in range(LC // TSC):
                        t0 = ht * TSC
                        da = scn.tile([DIN, TSC], F32, name="da2", tag="da",
                                      bufs=2)
                        nc.scalar.activation(da[:], dt_t[:, t0:t0 + TSC], AF.Exp,
                                             scale=a_s[:, s:s + 1])
                        db = scn.tile([DIN, TSC], F32, name="db2", tag="db",
                                      bufs=2)
                        nc.sync.dma_start(db[:], db_spill[s, :, t0:t0 + TSC])
                        h2 = scn.tile([DIN, TSC], F32, name="h2", tag="h1",
                                      bufs=2)
                        init = hin[:, s:s + 1] if ht == 0 else \
                            prev2[:, TSC - 1:TSC]
                        nc.vector.tensor_tensor_scan(h2[:], da[:], db[:], init,
                                                     OP.mult, OP.add)
                        prev2 = h2
                        hc = scn.tile([DIN, TSC], F32R, name="hc", tag="da",
                                      bufs=2)
                        nc.vector.tensor_tensor(hc[:], h2[:], bcc[:, t0:t0 + TSC],
                                                OP.mult)
                        for q in range(TSC // NSL):
                            nc.tensor.matmul(
                                ypsum[:, t0 + q * NSL:t0 + (q + 1) * NSL],
                                _r(id128_s[:]), _r(hc[:, q * NSL:(q + 1) * NSL]),
                                start=(s == 0), stop=(s == DST - 1))

                # y = (scan + u*D) * silu(z)  (into u_t)
                nc.vector.scalar_tensor_tensor(u_t[:], u_t[:], dvec_s[:],
                                               ypsum[:], OP.mult, OP.add)
            nc.vector.tensor_tensor(u_t[:], u_t[:], zs_t[:], OP.mult)

            # m = opt^T @ y ; out = (conv_out + 1) * m  (into co_t)
            with tc.tile_pool(name="mpp", bufs=1, space="PSUM") as mpp:
                mps = mpp.tile([C, LC], F32, name="mps")
                for sl0 in range(0, LC, NSL):
                    nc.tensor.matmul(mps[:, sl0:sl0 + NSL], _r(opt_s[:]),
                                     _r(u_t[:, sl0:sl0 + NSL]),
                                     start=True, stop=True)
                nc.vector.tensor_scalar(co_t[:], co_t[:], 1.0, None, OP.add)
                nc.vector.tensor_tensor(co_t[:], co_t[:], mps[:], OP.mult)
            nc.sync.dma_start(y_out[:], co_t[:])

        seq.release()
        cst.release()

    nc.compile()
    return nc


def _prep(inputs):
    x = np.asarray(inputs["x"], np.float32)
    conv1_w = np.asarray(inputs["conv1_w"], np.float32)
    conv1_b = np.asarray(inputs["conv1_b"], np.float32)
    conv2_w = np.asarray(inputs["conv2_w"], np.float32)
    conv2_b = np.asarray(inputs["conv2_b"], np.float32)
    ln_g = np.asarray(inputs["ln_g"], np.float32)
    ln_b = np.asarray(inputs["ln_b"], np.float32)
    in_proj_w = np.asarray(inputs["in_proj_w"], np.float32)
    conv1d_w = np.asarray(inputs["conv1d_w"], np.float32)
    conv1d_b = np.asarray(inputs["conv1d_b"], np.float32)
    x_proj_w = np.asarray(inputs["x_proj_w"], np.float32)
    dt_proj_w = np.asarray(inputs["dt_proj_w"], np.float32)
    dt_proj_b = np.asarray(inputs["dt_proj_b"], np.float32)
    A_log = np.asarray(inputs["A_log"], np.float32)
    D = np.asarray(inputs["D"], np.float32)
    out_proj_w = np.asarray(inputs["out_proj_w"], np.float32)

    def conv_t(wt):
        # (O, I, 3, 3) -> [I, tap*O], tap = ky*3+kx
        return np.ascontiguousarray(
            wt.transpose(2, 3, 1, 0).reshape(9, C, C).transpose(1, 0, 2)
            .reshape(C, 9 * C))

    wg = in_proj_w * ln_g[None, :]
    c2 = in_proj_w @ ln_b
    c2x = c2[:DIN]
    cwm = conv1d_w[:, 0, :]

    base = {
        "w1t": conv_t(conv1_w), "w2t": conv_t(conv2_w),
        "cb1": conv1_b.reshape(C, 1), "cb2": conv2_b.reshape(C, 1),
        "ident64": np.eye(C, dtype=np.float32),
        "ident128": np.eye(DIN, dtype=np.float32),
        # col 0: sum over co rows (64:128); col 1: sum over squares (0:64)
        "onesab": np.concatenate(
            [np.concatenate([np.zeros((C, 1)), np.ones((C, 1))], 1),
             np.concatenate([np.ones((C, 1)), np.zeros((C, 1))], 1)], 0),
        "wgt": np.concatenate([np.zeros((C, 2 * DIN), np.float32),
                               np.ascontiguousarray(wg.T)], 0),
        "xpwt": np.ascontiguousarray(x_proj_w.T),
        "dtwt": np.concatenate([np.zeros((32, DIN), np.float32),
                                np.ascontiguousarray(dt_proj_w.T)], 0),
        "dtb": dt_proj_b.reshape(DIN, 1),
        "cw": cwm,
        "bprime": (conv1d_b + c2x * cwm.sum(axis=1)).reshape(DIN, 1),
        "c2z": c2[DIN:].reshape(DIN, 1),
        "opt_w": np.ascontiguousarray(out_proj_w.T),
        "a_mat": -np.exp(A_log),
        "dvec": D.reshape(DIN, 1),
    }
    base = {k: np.ascontiguousarray(v, dtype=np.float32) for k, v in base.items()}

    in_maps = []
    for k in range(N_CORES):
        b, seg = divmod(k, SEGS)
        r0 = seg * ROWS
        xsl = np.zeros((C, XROWS, W), np.float32)
        lo, hi = r0 - 3, r0 + ROWS + 2
        slo, shi = max(lo, 0), min(hi, H)
        xsl[:, slo - lo:shi - lo, :] = x[b, :, slo:shi, :]
        al = np.zeros((N_CORES,), np.float32)
        al[SEGS * b:SEGS * b + seg] = 1.0
        m = {**base, "xs": xsl,
             "alpha": np.tile(al, (DIN, 1)),
             "maskc": np.full((DIN, 1), 0.0 if seg == 0 else 1.0, np.float32),
             "halo_fill": (np.tile((-c2x).reshape(DIN, 1), (1, 3))
                           if seg == 0 else np.zeros((DIN, 3), np.float32)),
             "c1m": np.tile(np.array(
                 [1.0 if 0 <= r0 - 2 + i < H else 0.0
                  for i in range(C1ROWS)], np.float32), (C, 1))}
        in_maps.append({kk: np.ascontiguousarray(vv, np.float32)
                        for kk, vv in m.items()})
    return in_maps


def kernel(**inputs):
    if "nc" not in _cached:
        _cached["nc"] = _build()
    nc = _cached["nc"]
    in_maps = _prep(inputs)
    res = run_bass_kernel_spmd(nc, in_maps, core_ids=list(range(N_CORES)))
    out = np.zeros((B, C, H, W), np.float32)
    for k in range(N_CORES):
        b, seg = divmod(k, SEGS)
        out[b, :, seg * ROWS:(seg + 1) * ROWS, :] = \
            res.results[k]["y_out"].reshape(C, ROWS, W)
    return out
